# revision 37
# baseline (speedup 1.0000x reference)
"""Trainium2 Bass kernel for nn_MultiHeadCrossAttention_57638461112647.

Sharding: 8 cores = 2 batches x 4-way split over attention *keys* (and,
identically, over output tokens). The softmax in the reference is over the
*query* axis (axis=1), so with scores laid out (keys on partitions, queries on
free) the softmax denominator Z[k] is a free-axis row-sum, fully core-local.
The only cross-core exchange is one fp8 ReduceScatter of the attention
output partials x^T = V'^T E (48 x 4096) within each 4-core batch group
(fp8e4m3 partials measured 1.7e-3 output err vs the 2e-2 gate).

Host-side prep (cheap, O(N*C^2) ~ 1.5% of total FLOPs, all constant-weight
linear layers): fold BN-as-affine + biases into the 1x1-conv weights, add the
constant 3D positional encodings, compute the token-wise projections
S1/Y1/Q/K/V in fp32, fold the attention scale into Q, fold the 3x3x3-conv
bias through the following 1x1 conv, pad the conv input with its halo, and
slice per-core chunks. The device keeps the O(N^2) attention and the 3x3x3
conv (98.5% of FLOPs).

Device pipeline per core (channel-major layouts, channels on partitions):
  scores^T = K_own^T Q_all          (PE, 8 k-tiles of 128 x 4096 q, bf16)
  E = exp(scores) bf16              (ACT; no max-sub needed: scores O(0.2);
                                     Z[k] fused via accum_out per tile)
  V' = V/Z bf16 (DVE, per 4-tile group as Z completes)
  x^T partial = sum_kt V'[kt]^T E[kt]  (PE, bf16, kt-outer shares LDWEIGHTS)
  ReduceScatter(4-core group, fp8) -> x^T reduced for own 1024 tokens
  conv3x3x3 as 27 accumulating bf16 matmuls on a padded slab + 1x1 -> Y2
  (conv fills the RS wait; taps share LDWEIGHTS across both 512-chunks)
  OUT rows 0-47: relu(Wo x^T + b) * (S+pe)[own chunk]; rows 48-95: Y2
Inputs arrive as 4 packed DMAs split across the two HWDGE rings (sync +
act); exp table preloaded via a dummy activation during input DMA; a tiny
warmup ReduceScatter fires under the exp phase to pre-heat the ncfw channel
before the real collective; post-exp casts/relu run on the then-idle ACT
engine in parallel with DVE.
"""
import numpy as np
import ml_dtypes
import jax
from jax.sharding import Mesh, PartitionSpec
from jax.experimental.shard_map import shard_map

import concourse.bass as bass
import concourse.mybir as mybir
import concourse.tile as tile
from concourse import bacc
from concourse import bass2jax
from concourse.bass2jax import _bass_exec_p, install_neuronx_cc_hook

F32 = mybir.dt.float32
BF16 = mybir.dt.bfloat16
FP8 = mybir.dt.float8e4
AF = mybir.ActivationFunctionType
AX = mybir.AxisListType

B, Cy, Cs, D, H, W = 2, 96, 48, 16, 16, 16
N = D * H * W            # 4096 tokens
NC = 8                   # cores
G = 4                    # cores per batch
KC = N // G              # keys / output tokens per core = 1024
KT = KC // 128           # k-tiles per core = 8
QC = 512                 # free-dim chunk per matmul
SFD = 2048               # scores psum tile free dim (4 PSUM banks)
EPS = 1e-5

_cache = {}


# ---------------------------------------------------------------- host prep
def _pe3d(C, x, y, z):
    """Transcription of reference.pe3d (incl. its quirky torch broadcasting)."""
    c = int(np.ceil(C / 3))
    inv_freq = (1.0 / (10000.0 ** (np.arange(0, c, 2, dtype=np.float32) / c))
                ).astype(np.float32)

    def emb(n):
        s = np.arange(n, dtype=np.float32)[:, None] * inv_freq[None, :]
        return np.concatenate([np.sin(s), np.cos(s)], axis=-1).astype(np.float32)

    out = np.zeros((x, y, z, 3 * c), np.float32)
    out[..., :c] = emb(x)[:, None, :]        # broadcasts against (y, z, c)
    out[..., c:2 * c] = emb(y)[:, None, :]
    out[..., 2 * c:3 * c] = emb(z)
    return np.ascontiguousarray(out[..., :C].transpose(3, 0, 1, 2))  # (C,x,y,z)


def _prepare(inputs):
    f = lambda a: np.ascontiguousarray(np.asarray(a, np.float32))
    bf = lambda a: np.ascontiguousarray(np.asarray(a, ml_dtypes.bfloat16))
    Y, S = f(inputs['Y']), f(inputs['S'])

    pe_s = _pe3d(Cs, D, H, W).reshape(Cs, N)
    pe_y = _pe3d(Cy, D, H, W).reshape(Cy, N)
    Scm = S.reshape(B, Cs, N) + pe_s[None]          # (B,48,4096)
    Ycm = Y.reshape(B, Cy, N) + pe_y[None]          # (B,96,4096)

    sb = lambda g: f(g) / np.sqrt(np.float32(1.0) + np.float32(EPS))

    def fold(w, b, g, be):
        s = sb(g)
        return f(w) * s[:, None], (f(b) * s + f(be)).astype(np.float32)

    WsF, bsF = fold(inputs['w_s'], inputs['b_s'], inputs['g_s'], inputs['be_s'])
    WyF, byF = fold(inputs['w_y'], inputs['b_y'], inputs['g_y'], inputs['be_y'])
    WoF, boF = fold(inputs['w_o'], inputs['b_o'], inputs['g_o'], inputs['be_o'])
    Wy2F, by2F = fold(inputs['w_y2'], inputs['b_y2'], inputs['g_y2'], inputs['be_y2'])
    by2FF = (Wy2F @ f(inputs['b3']) + by2F).astype(np.float32)

    # token-wise projections in fp32 on host (constant weights, ~1.5% of FLOPs)
    S1 = np.maximum(WsF @ Scm + bsF[None, :, None], 0.0)   # (B,48,4096)
    Y1 = np.maximum(WyF @ Ycm + byF[None, :, None], 0.0)   # (B,48,4096)
    scale = np.float32(Cs) ** np.float32(-0.5)
    Qt = np.einsum('de,bdn->ben', f(inputs['Wq']) * scale, Y1)   # (B,48,4096)
    Kt = np.einsum('de,bdn->ben', f(inputs['Wk']), Y1)           # (B,48,4096)
    Vt = np.einsum('de,bdn->ben', f(inputs['Wv']), S1)           # (B,48,4096)

    c = np.ascontiguousarray
    W3p = bf(f(inputs['w3']).reshape(Cy, Cy, 27).transpose(1, 2, 0))  # (i,t,o)
    B2 = c(np.stack([boF, by2FF], axis=1))                            # (48,2)

    Ypad = np.zeros((B, Cy, D + 2, H + 2, W + 2), np.float32)
    Ypad[:, :, 1:-1, 1:-1, 1:-1] = Ycm.reshape(B, Cy, D, H, W)

    # DW: w3 taps flat | Wy2F^T | WoF^T (padded to 96 rows)
    wo_pad = np.zeros((Cy, Cs), np.float32)
    wo_pad[:Cs] = WoF.T
    DW = bf(np.concatenate(
        [np.asarray(W3p, np.float32).reshape(Cy, 27 * Cy), Wy2F.T, wo_pad], axis=1))

    in_maps = []
    for core in range(NC):
        b, g = divmod(core, G)
        d0 = g * (D // G)
        sl = slice(g * KC, (g + 1) * KC)
        # D48: k | q | skb on 48 partitions, bf16
        D48 = bf(np.concatenate(
            [f(Kt[b, :, sl]), f(Qt[b]), f(Scm[b, :, sl])], axis=1))
        # DF: V flat [128, KT*Cs] | b2 [0:48, 384:386] f32
        Vk = c(Vt[b, :, sl].T.reshape(KT, 128, Cs).transpose(1, 0, 2))
        DF = np.zeros((128, KT * Cs + 2), np.float32)
        DF[:, :KT * Cs] = Vk.reshape(128, KT * Cs)
        DF[:Cs, KT * Cs:] = B2
        in_maps.append(dict(
            D48=D48,                                   # (48, 6144) bf16
            DF=c(DF),                                  # (128, 386) f32
            Yslab=bf(Ypad[b, :, d0:d0 + 6, :, :]),     # (96,6,18,18)
            DW=DW,                                     # (96, 2688) bf16
        ))
    return in_maps


# ---------------------------------------------------------------- bass build
def _build(repeat=1, ablate=()):
    """ablate: subset of {'rs','attn','conv'} — timing bisection only."""
    nc = bacc.Bacc("TRN2", target_bir_lowering=False, debug=False, num_devices=NC)

    D48 = nc.dram_tensor("D48", [Cs, KC + N + KC], BF16, kind="ExternalInput")
    DF = nc.dram_tensor("DF", [128, KT * Cs + 2], F32, kind="ExternalInput")
    Yslab = nc.dram_tensor("Yslab", [Cy, 6, 18, 18], BF16, kind="ExternalInput")
    DW = nc.dram_tensor("DW", [Cy, 27 * Cy + 2 * Cs], BF16, kind="ExternalInput")
    OUT = nc.dram_tensor("OUT", [2 * Cs, KC], F32, kind="ExternalOutput")

    with tile.TileContext(nc) as tc:
        with (
            tc.tile_pool(name="const", bufs=1) as cp,
            tc.tile_pool(name="data", bufs=1) as dp,
            tc.tile_pool(name="psum", bufs=2, space="PSUM") as pp,
            tc.tile_pool(name="dram", bufs=1, space="DRAM") as dram,
        ):
            def load(pool, t, shape, dt, tag=None, eng=None):
                s = pool.tile(shape, dt, tag=tag or t.name)
                (eng or nc.sync).dma_start(s[:], t.ap())
                return s

            # conv consts (on the ACT hwdge ring; sync ring carries D48)
            dw = load(cp, DW, [Cy, 27 * Cy + 2 * Cs], BF16, eng=nc.scalar)
            w3 = dw  # w3 tap t = dw[:, t*Cy:(t+1)*Cy]
            wy2 = dw[:, 27 * Cy:27 * Cy + Cs]           # (96,48)
            wo = dw[0:Cs, 27 * Cy + Cs:27 * Cy + 2 * Cs]  # (48,48)

            for rep in range(repeat):
                d48 = dp.tile([Cs, KC + N + KC], BF16, tag="d48")
                nc.sync.dma_start(d48[:, 0:KC + N // 2], D48.ap()[:, 0:KC + N // 2])
                nc.sync.dma_start(d48[:, KC + N // 2:], D48.ap()[:, KC + N // 2:])
                df = load(dp, DF, [128, KT * Cs + 2], F32, "df", eng=nc.scalar)
                # prime the ACT exp table while DMAs stream in
                scr = dp.tile([Cs, 2], F32, tag="scr")
                nc.scalar.activation(scr[:], df[0:Cs, KT * Cs:], AF.Exp)
                yslab = load(dp, Yslab, [Cy, 6, 18, 18], BF16, "yslab",
                             eng=nc.scalar)

                e = dp.tile([128, KT, N], BF16, tag="e")
                z2 = dp.tile([128, KT, 4], F32, tag="z2")
                z = dp.tile([128, KT], F32, tag="z")
                zr = dp.tile([128, KT], F32, tag="zr")
                vp = dp.tile([128, KT, Cs], BF16, tag="vp")
                xstage = dp.tile([Cs, N], BF16, tag="xstage")

                if 'rs' not in ablate:
                    # tiny warmup RS: pre-heats the ncfw channel/credits under
                    # the exp phase so the real RS's control-plane is shorter
                    win = dram.tile([G * Cs, 4], FP8, tag="win")
                    wout = dram.tile([Cs, 4], FP8, tag="wout")
                    wsb = dp.tile([Cs, 4], FP8, tag="wsb")
                    nc.gpsimd.memset(wsb[:], 0.0)
                    for gg in range(G):
                        nc.sync.dma_start(win[gg * Cs:(gg + 1) * Cs, :], wsb[:])
                    nc.gpsimd.collective_compute(
                        "ReduceScatter", mybir.AluOpType.add,
                        replica_groups=[[0, 1, 2, 3], [4, 5, 6, 7]],
                        ins=[win[:]], outs=[wout[:]],
                    )

                attn_on = 'attn' not in ablate
                if not attn_on:
                    nc.gpsimd.memset(e[:], 0.25)
                    nc.gpsimd.memset(vp[:], 0.5)

                # ---- scores + exp + Z per k-tile (k-tile 0 uses narrower
                # psum tiles so the first exp starts sooner)
                for kt in range(KT if attn_on else 0):
                    lhs = d48[:, kt * 128:(kt + 1) * 128]
                    fd = 1024 if kt == 0 else SFD
                    for h in range(N // fd):
                        pss = pp.tile([128, fd], F32, tag="ps")
                        for j in range(fd // QC):
                            qs = slice(KC + h * fd + j * QC,
                                       KC + h * fd + (j + 1) * QC)
                            nc.tensor.matmul(pss[:, j * QC:(j + 1) * QC],
                                             lhs, d48[:, qs], start=True, stop=True)
                        nc.scalar.activation(
                            e[:, kt, h * fd:(h + 1) * fd], pss[:], AF.Exp,
                            accum_out=z2[:, kt, h:h + 1])
                    nc.vector.tensor_add(z[:, kt:kt + 1], z2[:, kt, 0:1],
                                         z2[:, kt, 1:2])
                    for h2 in range(2, N // fd):
                        nc.vector.tensor_add(z[:, kt:kt + 1], z[:, kt:kt + 1],
                                             z2[:, kt, h2:h2 + 1])
                    if kt == KT // 2 - 1 or kt == KT - 1:
                        kts = range(0, KT // 2) if kt < KT - 1 else range(KT // 2, KT)
                        nc.vector.reciprocal(zr[:, kts.start:kts.stop],
                                             z[:, kts.start:kts.stop])
                        for k2 in kts:
                            nc.vector.tensor_scalar_mul(
                                vp[:, k2, :], df[:, k2 * Cs:(k2 + 1) * Cs],
                                zr[:, k2:k2 + 1])

                # ---- x^T partials (kt-outer shares V' LDWEIGHTS; two
                # 2048-wide halves in one PSUM slot each)
                cin = dram.tile([G * Cs, KC], FP8, tag="cin")
                cout = dram.tile([Cs, KC], FP8, tag="cout")
                xs = dp.tile([Cs, N], FP8, tag="xs")
                for half in range(2):
                    psx = pp.tile([128, SFD], F32, tag="ps")
                    for kt in range(KT):
                        for j in range(SFD // QC):
                            es = slice(half * SFD + j * QC,
                                       half * SFD + (j + 1) * QC)
                            nc.tensor.matmul(psx[0:Cs, j * QC:(j + 1) * QC],
                                             vp[:, kt, :], e[:, kt, es],
                                             start=(kt == 0), stop=(kt == KT - 1))
                    if half == 0:
                        nc.scalar.copy(xs[:, half * SFD:(half + 1) * SFD],
                                       psx[0:Cs, :])
                    else:
                        nc.vector.tensor_copy(
                            xs[:, half * SFD:(half + 1) * SFD], psx[0:Cs, :])
                    for gg in range(2):
                        g4 = half * 2 + gg
                        eng = nc.sync if gg == 0 else nc.scalar
                        eng.dma_start(cin[g4 * Cs:(g4 + 1) * Cs, :],
                                      xs[:, g4 * KC:(g4 + 1) * KC])
                if 'rs' not in ablate:
                    nc.gpsimd.collective_compute(
                        "ReduceScatter", mybir.AluOpType.add,
                        replica_groups=[[0, 1, 2, 3], [4, 5, 6, 7]],
                        ins=[cin[:]], outs=[cout[:]],
                    )
                else:
                    nc.sync.dma_start(cout[:], cin[0:Cs, :])

                # ---- conv3x3x3 + Y2 (overlaps the RS waits; PE-only + DVE).
                # Both 512-token chunks share each tap's LDWEIGHTS: one
                # [128,2048] psum slot holds both accumulation regions.
                if 'conv' not in ablate:
                    psc = pp.tile([128, SFD], F32, tag="ps")
                    for t in range(27):
                        kd, r = divmod(t, 9)
                        kh, kw = divmod(r, 3)
                        for ci in range(2):
                            nc.tensor.matmul(
                                psc[0:Cy, ci * QC:(ci + 1) * QC],
                                w3[:, t * Cy:(t + 1) * Cy],
                                yslab[:, 2 * ci + kd:2 * ci + kd + 2,
                                      kh:kh + 16, kw:kw + 16],
                                start=(t == 0), stop=(t == 26))
                    c3 = dp.tile([Cy, KC], BF16, tag="c3")
                    nc.vector.tensor_copy(c3[:], psc[0:Cy, 0:KC])
                    psy = pp.tile([128, SFD], F32, tag="ps")
                    for ci in range(2):
                        nc.tensor.matmul(psy[0:Cs, ci * QC:(ci + 1) * QC],
                                         wy2,
                                         c3[:, ci * QC:(ci + 1) * QC],
                                         start=True, stop=True)
                    y2 = dp.tile([Cs, KC], F32, tag="y2")
                    nc.vector.tensor_scalar(y2[:], psy[0:Cs, 0:KC],
                                            df[0:Cs, KT * Cs + 1:KT * Cs + 2], 0.0,
                                            mybir.AluOpType.add,
                                            mybir.AluOpType.max)
                    nc.scalar.dma_start(OUT.ap()[Cs:2 * Cs, :], y2[:])

                # ---- post-RS: out-projection, mul by S+pe (xr halves on
                # both rings so proj[0] starts before half 1 lands)
                xr8 = dp.tile([Cs, KC], FP8, tag="xr8")
                nc.sync.dma_start(xr8[:, 0:QC], cout[0:Cs, 0:QC])
                nc.scalar.dma_start(xr8[:, QC:], cout[0:Cs, QC:])
                xr = dp.tile([Cs, KC], BF16, tag="xr")
                zo = dp.tile([Cs, KC], F32, tag="zo")
                for half in range(2):
                    sl = slice(half * QC, (half + 1) * QC)
                    if half == 0:
                        nc.scalar.copy(xr[:, sl], xr8[:, sl])
                    else:
                        nc.vector.tensor_copy(xr[:, sl], xr8[:, sl])
                    psz = pp.tile([128, SFD], F32, tag="ps")
                    nc.tensor.matmul(psz[0:Cs, 0:QC], wo, xr[:, sl],
                                     start=True, stop=True)
                    zc = dp.tile([Cs, QC], F32, tag=f"zc{half}")
                    nc.scalar.activation(zc[:], psz[0:Cs, 0:QC], AF.Relu,
                                         bias=df[0:Cs, KT * Cs:KT * Cs + 1])
                    nc.vector.tensor_mul(zo[:, sl], zc[:],
                                         d48[:, KC + N + half * QC:KC + N + (half + 1) * QC])
                    eng = nc.sync if half == 0 else nc.scalar
                    eng.dma_start(OUT.ap()[0:Cs, sl], zo[:, sl])

    nc.compile()
    return nc


class _Runner:
    """Builds the bass module once and a single reusable jitted callable
    (re-jitting per call would re-trace + re-hash the BIR module: ~600ms)."""

    def __init__(self, repeat=1, ablate=(), **kw):
        install_neuronx_cc_hook()
        nc = _build(repeat, ablate, **kw)
        self._setup_from_nc(nc)

    def _setup_from_nc(self, nc):
        install_neuronx_cc_hook()
        pid = nc.partition_id_tensor.name if nc.partition_id_tensor else None
        in_names, out_names, out_avals = [], [], []
        for alloc in nc.m.functions[0].allocations:
            if not isinstance(alloc, mybir.MemoryLocationSet):
                continue
            name = alloc.memorylocations[0].name
            if alloc.kind == "ExternalInput":
                if name != pid:
                    in_names.append(name)
            elif alloc.kind == "ExternalOutput":
                out_names.append(name)
                out_avals.append(jax.core.ShapedArray(
                    tuple(alloc.tensor_shape), mybir.dt.np(alloc.dtype)))
        self.in_names, self.out_names, self.out_avals = in_names, out_names, out_avals
        all_names = in_names + out_names + ([pid] if pid else [])

        def _body(*args):
            operands = list(args)
            if pid is not None:
                operands.append(bass2jax.partition_id_tensor())
            return tuple(_bass_exec_p.bind(
                *operands, out_avals=tuple(out_avals), in_names=tuple(all_names),
                out_names=tuple(out_names), lowering_input_output_aliases=(),
                sim_require_finite=True, sim_require_nnan=True, nc=nc))

        mesh = self.mesh = Mesh(np.asarray(jax.devices()[:NC]), ("core",))
        sp = (PartitionSpec("core"),)
        n_in = len(in_names) + len(out_names)
        self.fn = jax.jit(
            shard_map(_body, mesh=mesh, in_specs=sp * n_in,
                      out_specs=sp * len(out_names), check_rep=False),
            keep_unused=True)

    def device_args(self, in_maps):
        """Pre-stage all inputs on device (sharded) for low-overhead timed calls."""
        from jax.sharding import NamedSharding
        sh = NamedSharding(self.mesh, PartitionSpec("core"))
        cat = [np.concatenate([in_maps[c][n] for c in range(NC)], axis=0)
               for n in self.in_names]
        zz = [np.zeros((NC * a.shape[0], *a.shape[1:]), a.dtype)
              for a in self.out_avals]
        return [jax.device_put(a, sh) for a in cat + zz]

    def __call__(self, in_maps):
        outs = None
        for attempt in range(3):
            try:
                args = self.device_args(in_maps)
                jax.block_until_ready(args)  # staged before launch: less skew
                outs = self.fn(*args)
                jax.block_until_ready(outs)
                break
            except jax.errors.JaxRuntimeError:
                # transient tunnel hiccup (mesh desync / worker hangup):
                # restage and relaunch
                if attempt == 2:
                    raise
        assert outs is not None
        return [
            {n: np.asarray(outs[i]).reshape(NC, *self.out_avals[i].shape)[c]
             for i, n in enumerate(self.out_names)}
            for c in range(NC)
        ]


def _get(repeat=1, ablate=(), **kw):
    key = (repeat, tuple(sorted(ablate)), tuple(sorted(kw.items())))
    if key not in _cache:
        _cache[key] = _Runner(repeat, ablate, **kw)
    return _cache[key]


# ---------------------------------------------------------------- entry point
def kernel(**inputs):
    in_maps = _prepare(inputs)
    results = _get(1)(in_maps)
    out = np.zeros((B, 2 * Cs, D, H, W), np.float32)
    for core in range(NC):
        b, g = divmod(core, G)
        blk = results[core]["OUT"].reshape(2 * Cs, D // G, H, W)
        out[b, :, g * (D // G):(g + 1) * (D // G)] = blk
    return out


# revision 38
# speedup vs baseline: 1.0246x; 1.0246x over previous
"""Trainium2 Bass kernel for nn_MultiHeadCrossAttention_57638461112647.

Sharding: 8 cores = 2 batches x 4-way split over attention *keys* (and,
identically, over output tokens). The softmax in the reference is over the
*query* axis (axis=1), so with scores laid out (keys on partitions, queries on
free) the softmax denominator Z[k] is a free-axis row-sum, fully core-local.
The only cross-core exchange is one fp8 ReduceScatter of the attention
output partials x^T = V'^T E (48 x 4096) within each 4-core batch group
(fp8e4m3 partials measured 1.7e-3 output err vs the 2e-2 gate).

Host-side prep (cheap, O(N*C^2) ~ 1.5% of total FLOPs, all constant-weight
linear layers): fold BN-as-affine + biases into the 1x1-conv weights, add the
constant 3D positional encodings, compute the token-wise projections
S1/Y1/Q/K/V in fp32, fold the attention scale into Q, fold the 3x3x3-conv
bias through the following 1x1 conv, pad the conv input with its halo, and
slice per-core chunks. The device keeps the O(N^2) attention and the 3x3x3
conv (98.5% of FLOPs).

Device pipeline per core (channel-major layouts, channels on partitions):
  scores^T = K_own^T Q_all          (PE, 8 k-tiles of 128 x 4096 q, bf16)
  E = exp(scores) bf16              (ACT; no max-sub needed: scores O(0.2);
                                     Z[k] fused via accum_out per tile)
  V' = V/Z bf16 (DVE, per 4-tile group as Z completes)
  x^T partial = sum_kt V'[kt]^T E[kt]  (PE, bf16, kt-outer shares LDWEIGHTS)
  ReduceScatter(4-core group, fp8) -> x^T reduced for own 1024 tokens
  conv3x3x3 as 27 accumulating bf16 matmuls on a padded slab + 1x1 -> Y2
  (conv fills the RS wait; taps share LDWEIGHTS across both 512-chunks)
  OUT rows 0-47: relu(Wo x^T + b) * (S+pe)[own chunk]; rows 48-95: Y2
Inputs arrive as 4 packed DMAs split across the two HWDGE rings (sync +
act); exp table preloaded via a dummy activation during input DMA; a tiny
warmup ReduceScatter fires under the exp phase to pre-heat the ncfw channel
before the real collective; post-exp casts/relu run on the then-idle ACT
engine in parallel with DVE.
"""
import numpy as np
import ml_dtypes
import jax
from jax.sharding import Mesh, PartitionSpec
from jax.experimental.shard_map import shard_map

import concourse.bass as bass
import concourse.mybir as mybir
import concourse.tile as tile
from concourse import bacc
from concourse import bass2jax
from concourse.bass2jax import _bass_exec_p, install_neuronx_cc_hook

F32 = mybir.dt.float32
BF16 = mybir.dt.bfloat16
FP8 = mybir.dt.float8e4
AF = mybir.ActivationFunctionType
AX = mybir.AxisListType

B, Cy, Cs, D, H, W = 2, 96, 48, 16, 16, 16
N = D * H * W            # 4096 tokens
NC = 8                   # cores
G = 4                    # cores per batch
KC = N // G              # keys / output tokens per core = 1024
KT = KC // 128           # k-tiles per core = 8
QC = 512                 # free-dim chunk per matmul
SFD = 2048               # scores psum tile free dim (4 PSUM banks)
EPS = 1e-5

_cache = {}


# ---------------------------------------------------------------- host prep
def _pe3d(C, x, y, z):
    """Transcription of reference.pe3d (incl. its quirky torch broadcasting)."""
    c = int(np.ceil(C / 3))
    inv_freq = (1.0 / (10000.0 ** (np.arange(0, c, 2, dtype=np.float32) / c))
                ).astype(np.float32)

    def emb(n):
        s = np.arange(n, dtype=np.float32)[:, None] * inv_freq[None, :]
        return np.concatenate([np.sin(s), np.cos(s)], axis=-1).astype(np.float32)

    out = np.zeros((x, y, z, 3 * c), np.float32)
    out[..., :c] = emb(x)[:, None, :]        # broadcasts against (y, z, c)
    out[..., c:2 * c] = emb(y)[:, None, :]
    out[..., 2 * c:3 * c] = emb(z)
    return np.ascontiguousarray(out[..., :C].transpose(3, 0, 1, 2))  # (C,x,y,z)


def _prepare(inputs):
    f = lambda a: np.ascontiguousarray(np.asarray(a, np.float32))
    bf = lambda a: np.ascontiguousarray(np.asarray(a, ml_dtypes.bfloat16))
    Y, S = f(inputs['Y']), f(inputs['S'])

    pe_s = _pe3d(Cs, D, H, W).reshape(Cs, N)
    pe_y = _pe3d(Cy, D, H, W).reshape(Cy, N)
    Scm = S.reshape(B, Cs, N) + pe_s[None]          # (B,48,4096)
    Ycm = Y.reshape(B, Cy, N) + pe_y[None]          # (B,96,4096)

    sb = lambda g: f(g) / np.sqrt(np.float32(1.0) + np.float32(EPS))

    def fold(w, b, g, be):
        s = sb(g)
        return f(w) * s[:, None], (f(b) * s + f(be)).astype(np.float32)

    WsF, bsF = fold(inputs['w_s'], inputs['b_s'], inputs['g_s'], inputs['be_s'])
    WyF, byF = fold(inputs['w_y'], inputs['b_y'], inputs['g_y'], inputs['be_y'])
    WoF, boF = fold(inputs['w_o'], inputs['b_o'], inputs['g_o'], inputs['be_o'])
    Wy2F, by2F = fold(inputs['w_y2'], inputs['b_y2'], inputs['g_y2'], inputs['be_y2'])
    by2FF = (Wy2F @ f(inputs['b3']) + by2F).astype(np.float32)

    # token-wise projections in fp32 on host (constant weights, ~1.5% of FLOPs)
    S1 = np.maximum(WsF @ Scm + bsF[None, :, None], 0.0)   # (B,48,4096)
    Y1 = np.maximum(WyF @ Ycm + byF[None, :, None], 0.0)   # (B,48,4096)
    scale = np.float32(Cs) ** np.float32(-0.5)
    Qt = np.einsum('de,bdn->ben', f(inputs['Wq']) * scale, Y1)   # (B,48,4096)
    Kt = np.einsum('de,bdn->ben', f(inputs['Wk']), Y1)           # (B,48,4096)
    Vt = np.einsum('de,bdn->ben', f(inputs['Wv']), S1)           # (B,48,4096)

    c = np.ascontiguousarray
    W3p = bf(f(inputs['w3']).reshape(Cy, Cy, 27).transpose(1, 2, 0))  # (i,t,o)
    B2 = c(np.stack([boF, by2FF], axis=1))                            # (48,2)

    Ypad = np.zeros((B, Cy, D + 2, H + 2, W + 2), np.float32)
    Ypad[:, :, 1:-1, 1:-1, 1:-1] = Ycm.reshape(B, Cy, D, H, W)

    # DW: w3 taps flat | Wy2F^T | WoF^T (padded to 96 rows)
    wo_pad = np.zeros((Cy, Cs), np.float32)
    wo_pad[:Cs] = WoF.T
    DW = bf(np.concatenate(
        [np.asarray(W3p, np.float32).reshape(Cy, 27 * Cy), Wy2F.T, wo_pad], axis=1))

    in_maps = []
    for core in range(NC):
        b, g = divmod(core, G)
        d0 = g * (D // G)
        sl = slice(g * KC, (g + 1) * KC)
        # D48: k | q | skb on 48 partitions, bf16
        D48 = bf(np.concatenate(
            [f(Kt[b, :, sl]), f(Qt[b]), f(Scm[b, :, sl])], axis=1))
        # DF: V flat [128, KT*Cs] | b2 [0:48, 384:386] f32
        Vk = c(Vt[b, :, sl].T.reshape(KT, 128, Cs).transpose(1, 0, 2))
        DF = np.zeros((128, KT * Cs + 2), np.float32)
        DF[:, :KT * Cs] = Vk.reshape(128, KT * Cs)
        DF[:Cs, KT * Cs:] = B2
        in_maps.append(dict(
            D48=D48,                                   # (48, 6144) bf16
            DF=c(DF),                                  # (128, 386) f32
            Yslab=bf(Ypad[b, :, d0:d0 + 6, :, :]),     # (96,6,18,18)
            DW=DW,                                     # (96, 2688) bf16
        ))
    return in_maps


# ---------------------------------------------------------------- bass build
def _build(repeat=1, ablate=()):
    """ablate: subset of {'rs','attn','conv'} — timing bisection only."""
    nc = bacc.Bacc("TRN2", target_bir_lowering=False, debug=False, num_devices=NC)

    D48 = nc.dram_tensor("D48", [Cs, KC + N + KC], BF16, kind="ExternalInput")
    DF = nc.dram_tensor("DF", [128, KT * Cs + 2], F32, kind="ExternalInput")
    Yslab = nc.dram_tensor("Yslab", [Cy, 6, 18, 18], BF16, kind="ExternalInput")
    DW = nc.dram_tensor("DW", [Cy, 27 * Cy + 2 * Cs], BF16, kind="ExternalInput")
    OUT = nc.dram_tensor("OUT", [2 * Cs, KC], F32, kind="ExternalOutput")

    with tile.TileContext(nc) as tc:
        with (
            tc.tile_pool(name="const", bufs=1) as cp,
            tc.tile_pool(name="data", bufs=1) as dp,
            tc.tile_pool(name="psum", bufs=2, space="PSUM") as pp,
            tc.tile_pool(name="dram", bufs=1, space="DRAM") as dram,
        ):
            def load(pool, t, shape, dt, tag=None, eng=None):
                s = pool.tile(shape, dt, tag=tag or t.name)
                (eng or nc.sync).dma_start(s[:], t.ap())
                return s

            # conv consts (on the ACT hwdge ring; sync ring carries D48)
            dw = load(cp, DW, [Cy, 27 * Cy + 2 * Cs], BF16, eng=nc.scalar)
            w3 = dw  # w3 tap t = dw[:, t*Cy:(t+1)*Cy]
            wy2 = dw[:, 27 * Cy:27 * Cy + Cs]           # (96,48)
            wo = dw[0:Cs, 27 * Cy + Cs:27 * Cy + 2 * Cs]  # (48,48)

            for rep in range(repeat):
                d48 = dp.tile([Cs, KC + N + KC], BF16, tag="d48")
                nc.sync.dma_start(d48[:, 0:KC + N // 2], D48.ap()[:, 0:KC + N // 2])
                nc.sync.dma_start(d48[:, KC + N // 2:], D48.ap()[:, KC + N // 2:])
                df = load(dp, DF, [128, KT * Cs + 2], F32, "df", eng=nc.scalar)
                # prime the ACT exp table while DMAs stream in
                scr = dp.tile([Cs, 2], F32, tag="scr")
                nc.scalar.activation(scr[:], df[0:Cs, KT * Cs:], AF.Exp)
                yslab = load(dp, Yslab, [Cy, 6, 18, 18], BF16, "yslab",
                             eng=nc.scalar)

                e = dp.tile([128, KT, N], BF16, tag="e")
                z2 = dp.tile([128, KT, 4], F32, tag="z2")
                z = dp.tile([128, KT], F32, tag="z")
                zr = dp.tile([128, KT], F32, tag="zr")
                vp = dp.tile([128, KT, Cs], BF16, tag="vp")
                xstage = dp.tile([Cs, N], BF16, tag="xstage")

                if 'rs' not in ablate:
                    # tiny warmup RS: pre-heats the ncfw channel/credits under
                    # the exp phase so the real RS's control-plane is shorter
                    win = dram.tile([G * Cs, 4], FP8, tag="win")
                    wout = dram.tile([Cs, 4], FP8, tag="wout")
                    wsb = dp.tile([Cs, 4], FP8, tag="wsb")
                    nc.gpsimd.memset(wsb[:], 0.0)
                    for gg in range(G):
                        nc.sync.dma_start(win[gg * Cs:(gg + 1) * Cs, :], wsb[:])
                    nc.gpsimd.collective_compute(
                        "ReduceScatter", mybir.AluOpType.add,
                        replica_groups=[[0, 1, 2, 3], [4, 5, 6, 7]],
                        ins=[win[:]], outs=[wout[:]],
                    )

                attn_on = 'attn' not in ablate
                if not attn_on:
                    nc.gpsimd.memset(e[:], 0.25)
                    nc.gpsimd.memset(vp[:], 0.5)

                # ---- scores + exp + Z per k-tile
                for kt in range(KT if attn_on else 0):
                    lhs = d48[:, kt * 128:(kt + 1) * 128]
                    for h in range(N // SFD):
                        pss = pp.tile([128, SFD], F32, tag="ps")
                        for j in range(SFD // QC):
                            qs = slice(KC + h * SFD + j * QC,
                                       KC + h * SFD + (j + 1) * QC)
                            nc.tensor.matmul(pss[:, j * QC:(j + 1) * QC],
                                             lhs, d48[:, qs], start=True, stop=True)
                        nc.scalar.activation(
                            e[:, kt, h * SFD:(h + 1) * SFD], pss[:], AF.Exp,
                            accum_out=z2[:, kt, h:h + 1])
                    nc.vector.tensor_add(z[:, kt:kt + 1], z2[:, kt, 0:1],
                                         z2[:, kt, 1:2])
                    if kt == KT // 2 - 1 or kt == KT - 1:
                        kts = range(0, KT // 2) if kt < KT - 1 else range(KT // 2, KT)
                        nc.vector.reciprocal(zr[:, kts.start:kts.stop],
                                             z[:, kts.start:kts.stop])
                        for k2 in kts:
                            nc.vector.tensor_scalar_mul(
                                vp[:, k2, :], df[:, k2 * Cs:(k2 + 1) * Cs],
                                zr[:, k2:k2 + 1])

                # ---- x^T partials (kt-outer shares V' LDWEIGHTS; two
                # 2048-wide halves in one PSUM slot each)
                cin = dram.tile([G * Cs, KC], FP8, tag="cin")
                cout = dram.tile([Cs, KC], FP8, tag="cout")
                xs = dp.tile([Cs, N], FP8, tag="xs")
                for half in range(2):
                    psx = pp.tile([128, SFD], F32, tag="ps")
                    for kt in range(KT):
                        for j in range(SFD // QC):
                            es = slice(half * SFD + j * QC,
                                       half * SFD + (j + 1) * QC)
                            nc.tensor.matmul(psx[0:Cs, j * QC:(j + 1) * QC],
                                             vp[:, kt, :], e[:, kt, es],
                                             start=(kt == 0), stop=(kt == KT - 1))
                    if half == 0:
                        nc.scalar.copy(xs[:, half * SFD:(half + 1) * SFD],
                                       psx[0:Cs, :])
                    else:
                        nc.vector.tensor_copy(
                            xs[:, half * SFD:(half + 1) * SFD], psx[0:Cs, :])
                    for gg in range(2):
                        g4 = half * 2 + gg
                        eng = nc.sync if gg == 0 else nc.scalar
                        eng.dma_start(cin[g4 * Cs:(g4 + 1) * Cs, :],
                                      xs[:, g4 * KC:(g4 + 1) * KC])
                if 'rs' not in ablate:
                    nc.gpsimd.collective_compute(
                        "ReduceScatter", mybir.AluOpType.add,
                        replica_groups=[[0, 1, 2, 3], [4, 5, 6, 7]],
                        ins=[cin[:]], outs=[cout[:]],
                    )
                else:
                    nc.sync.dma_start(cout[:], cin[0:Cs, :])

                # ---- conv3x3x3 + Y2 (overlaps the RS waits; PE-only + DVE).
                # Both 512-token chunks share each tap's LDWEIGHTS: one
                # [128,2048] psum slot holds both accumulation regions.
                if 'conv' not in ablate:
                    psc = pp.tile([128, SFD], F32, tag="ps")
                    for t in range(27):
                        kd, r = divmod(t, 9)
                        kh, kw = divmod(r, 3)
                        for ci in range(2):
                            nc.tensor.matmul(
                                psc[0:Cy, ci * QC:(ci + 1) * QC],
                                w3[:, t * Cy:(t + 1) * Cy],
                                yslab[:, 2 * ci + kd:2 * ci + kd + 2,
                                      kh:kh + 16, kw:kw + 16],
                                start=(t == 0), stop=(t == 26))
                    c3 = dp.tile([Cy, KC], BF16, tag="c3")
                    nc.vector.tensor_copy(c3[:], psc[0:Cy, 0:KC])
                    psy = pp.tile([128, SFD], F32, tag="ps")
                    for ci in range(2):
                        nc.tensor.matmul(psy[0:Cs, ci * QC:(ci + 1) * QC],
                                         wy2,
                                         c3[:, ci * QC:(ci + 1) * QC],
                                         start=True, stop=True)
                    y2 = dp.tile([Cs, KC], F32, tag="y2")
                    nc.vector.tensor_scalar(y2[:], psy[0:Cs, 0:KC],
                                            df[0:Cs, KT * Cs + 1:KT * Cs + 2], 0.0,
                                            mybir.AluOpType.add,
                                            mybir.AluOpType.max)
                    nc.scalar.dma_start(OUT.ap()[Cs:2 * Cs, :], y2[:])

                # ---- post-RS: out-projection, mul by S+pe (xr halves on
                # both rings so proj[0] starts before half 1 lands)
                xr8 = dp.tile([Cs, KC], FP8, tag="xr8")
                nc.sync.dma_start(xr8[:, 0:QC], cout[0:Cs, 0:QC])
                nc.scalar.dma_start(xr8[:, QC:], cout[0:Cs, QC:])
                xr = dp.tile([Cs, KC], BF16, tag="xr")
                zo = dp.tile([Cs, KC], F32, tag="zo")
                for half in range(2):
                    sl = slice(half * QC, (half + 1) * QC)
                    if half == 0:
                        nc.scalar.copy(xr[:, sl], xr8[:, sl])
                    else:
                        nc.vector.tensor_copy(xr[:, sl], xr8[:, sl])
                    psz = pp.tile([128, SFD], F32, tag="ps")
                    nc.tensor.matmul(psz[0:Cs, 0:QC], wo, xr[:, sl],
                                     start=True, stop=True)
                    zc = dp.tile([Cs, QC], F32, tag=f"zc{half}")
                    nc.scalar.activation(zc[:], psz[0:Cs, 0:QC], AF.Relu,
                                         bias=df[0:Cs, KT * Cs:KT * Cs + 1])
                    nc.vector.tensor_mul(zo[:, sl], zc[:],
                                         d48[:, KC + N + half * QC:KC + N + (half + 1) * QC])
                    eng = nc.sync if half == 0 else nc.scalar
                    eng.dma_start(OUT.ap()[0:Cs, sl], zo[:, sl])

    nc.compile()
    return nc


class _Runner:
    """Builds the bass module once and a single reusable jitted callable
    (re-jitting per call would re-trace + re-hash the BIR module: ~600ms)."""

    def __init__(self, repeat=1, ablate=(), **kw):
        install_neuronx_cc_hook()
        nc = _build(repeat, ablate, **kw)
        self._setup_from_nc(nc)

    def _setup_from_nc(self, nc):
        install_neuronx_cc_hook()
        pid = nc.partition_id_tensor.name if nc.partition_id_tensor else None
        in_names, out_names, out_avals = [], [], []
        for alloc in nc.m.functions[0].allocations:
            if not isinstance(alloc, mybir.MemoryLocationSet):
                continue
            name = alloc.memorylocations[0].name
            if alloc.kind == "ExternalInput":
                if name != pid:
                    in_names.append(name)
            elif alloc.kind == "ExternalOutput":
                out_names.append(name)
                out_avals.append(jax.core.ShapedArray(
                    tuple(alloc.tensor_shape), mybir.dt.np(alloc.dtype)))
        self.in_names, self.out_names, self.out_avals = in_names, out_names, out_avals
        all_names = in_names + out_names + ([pid] if pid else [])

        def _body(*args):
            operands = list(args)
            if pid is not None:
                operands.append(bass2jax.partition_id_tensor())
            return tuple(_bass_exec_p.bind(
                *operands, out_avals=tuple(out_avals), in_names=tuple(all_names),
                out_names=tuple(out_names), lowering_input_output_aliases=(),
                sim_require_finite=True, sim_require_nnan=True, nc=nc))

        mesh = self.mesh = Mesh(np.asarray(jax.devices()[:NC]), ("core",))
        sp = (PartitionSpec("core"),)
        n_in = len(in_names) + len(out_names)
        self.fn = jax.jit(
            shard_map(_body, mesh=mesh, in_specs=sp * n_in,
                      out_specs=sp * len(out_names), check_rep=False),
            keep_unused=True)

    def device_args(self, in_maps):
        """Pre-stage all inputs on device (sharded) for low-overhead timed calls."""
        from jax.sharding import NamedSharding
        sh = NamedSharding(self.mesh, PartitionSpec("core"))
        cat = [np.concatenate([in_maps[c][n] for c in range(NC)], axis=0)
               for n in self.in_names]
        zz = [np.zeros((NC * a.shape[0], *a.shape[1:]), a.dtype)
              for a in self.out_avals]
        return [jax.device_put(a, sh) for a in cat + zz]

    def __call__(self, in_maps):
        outs = None
        for attempt in range(3):
            try:
                args = self.device_args(in_maps)
                jax.block_until_ready(args)  # staged before launch: less skew
                outs = self.fn(*args)
                jax.block_until_ready(outs)
                break
            except jax.errors.JaxRuntimeError:
                # transient tunnel hiccup (mesh desync / worker hangup):
                # restage and relaunch
                if attempt == 2:
                    raise
        assert outs is not None
        return [
            {n: np.asarray(outs[i]).reshape(NC, *self.out_avals[i].shape)[c]
             for i, n in enumerate(self.out_names)}
            for c in range(NC)
        ]


def _get(repeat=1, ablate=(), **kw):
    key = (repeat, tuple(sorted(ablate)), tuple(sorted(kw.items())))
    if key not in _cache:
        _cache[key] = _Runner(repeat, ablate, **kw)
    return _cache[key]


# ---------------------------------------------------------------- entry point
def kernel(**inputs):
    in_maps = _prepare(inputs)
    results = _get(1)(in_maps)
    out = np.zeros((B, 2 * Cs, D, H, W), np.float32)
    for core in range(NC):
        b, g = divmod(core, G)
        blk = results[core]["OUT"].reshape(2 * Cs, D // G, H, W)
        out[b, :, g * (D // G):(g + 1) * (D // G)] = blk
    return out


# revision 39
# speedup vs baseline: 1.0462x; 1.0211x over previous
"""Trainium2 Bass kernel for nn_MultiHeadCrossAttention_57638461112647.

Sharding: 8 cores = 2 batches x 4-way split over attention *keys* (and,
identically, over output tokens). The softmax in the reference is over the
*query* axis (axis=1), so with scores laid out (keys on partitions, queries on
free) the softmax denominator Z[k] is a free-axis row-sum, fully core-local.
The only cross-core exchange is one fp8 ReduceScatter of the attention
output partials x^T = V'^T E (48 x 4096) within each 4-core batch group
(fp8e4m3 partials measured 1.7e-3 output err vs the 2e-2 gate).

Host-side prep (cheap, O(N*C^2) ~ 1.5% of total FLOPs, all constant-weight
linear layers): fold BN-as-affine + biases into the 1x1-conv weights, add the
constant 3D positional encodings, compute the token-wise projections
S1/Y1/Q/K/V in fp32, fold the attention scale into Q, fold the 3x3x3-conv
bias through the following 1x1 conv, pad the conv input with its halo, and
slice per-core chunks. The device keeps the O(N^2) attention and the 3x3x3
conv (98.5% of FLOPs).

Device pipeline per core (channel-major layouts, channels on partitions):
  scores^T = K_own^T Q_all          (PE, 8 k-tiles of 128 x 4096 q, bf16)
  E = exp(scores) bf16              (ACT; no max-sub needed: scores O(0.2);
                                     Z[k] fused via accum_out per tile)
  V' = V/Z bf16 (DVE, per 4-tile group as Z completes)
  x^T partial = sum_kt V'[kt]^T E[kt]  (PE, bf16, kt-outer shares LDWEIGHTS)
  ReduceScatter(4-core group, fp8) -> x^T reduced for own 1024 tokens
  conv3x3x3 as 27 accumulating bf16 matmuls on a padded slab + 1x1 -> Y2
  (conv fills the RS wait; taps share LDWEIGHTS across both 512-chunks)
  OUT rows 0-47: relu(Wo x^T + b) * (S+pe)[own chunk]; rows 48-95: Y2
Inputs arrive as 4 packed DMAs split across the two HWDGE rings (sync +
act); exp table preloaded via a dummy activation during input DMA; a tiny
warmup ReduceScatter fires under the exp phase to pre-heat the ncfw channel
before the real collective; post-exp casts/relu run on the then-idle ACT
engine in parallel with DVE.
"""
import numpy as np
import ml_dtypes
import jax
from jax.sharding import Mesh, PartitionSpec
from jax.experimental.shard_map import shard_map

import concourse.bass as bass
import concourse.mybir as mybir
import concourse.tile as tile
from concourse import bacc
from concourse import bass2jax
from concourse.bass2jax import _bass_exec_p, install_neuronx_cc_hook

F32 = mybir.dt.float32
BF16 = mybir.dt.bfloat16
FP8 = mybir.dt.float8e4
AF = mybir.ActivationFunctionType
AX = mybir.AxisListType

B, Cy, Cs, D, H, W = 2, 96, 48, 16, 16, 16
N = D * H * W            # 4096 tokens
NC = 8                   # cores
G = 4                    # cores per batch
KC = N // G              # keys / output tokens per core = 1024
KT = KC // 128           # k-tiles per core = 8
QC = 512                 # free-dim chunk per matmul
SFD = 2048               # scores psum tile free dim (4 PSUM banks)
EPS = 1e-5

_cache = {}


# ---------------------------------------------------------------- host prep
def _pe3d(C, x, y, z):
    """Transcription of reference.pe3d (incl. its quirky torch broadcasting)."""
    c = int(np.ceil(C / 3))
    inv_freq = (1.0 / (10000.0 ** (np.arange(0, c, 2, dtype=np.float32) / c))
                ).astype(np.float32)

    def emb(n):
        s = np.arange(n, dtype=np.float32)[:, None] * inv_freq[None, :]
        return np.concatenate([np.sin(s), np.cos(s)], axis=-1).astype(np.float32)

    out = np.zeros((x, y, z, 3 * c), np.float32)
    out[..., :c] = emb(x)[:, None, :]        # broadcasts against (y, z, c)
    out[..., c:2 * c] = emb(y)[:, None, :]
    out[..., 2 * c:3 * c] = emb(z)
    return np.ascontiguousarray(out[..., :C].transpose(3, 0, 1, 2))  # (C,x,y,z)


def _prepare(inputs):
    f = lambda a: np.ascontiguousarray(np.asarray(a, np.float32))
    bf = lambda a: np.ascontiguousarray(np.asarray(a, ml_dtypes.bfloat16))
    Y, S = f(inputs['Y']), f(inputs['S'])

    pe_s = _pe3d(Cs, D, H, W).reshape(Cs, N)
    pe_y = _pe3d(Cy, D, H, W).reshape(Cy, N)
    Scm = S.reshape(B, Cs, N) + pe_s[None]          # (B,48,4096)
    Ycm = Y.reshape(B, Cy, N) + pe_y[None]          # (B,96,4096)

    sb = lambda g: f(g) / np.sqrt(np.float32(1.0) + np.float32(EPS))

    def fold(w, b, g, be):
        s = sb(g)
        return f(w) * s[:, None], (f(b) * s + f(be)).astype(np.float32)

    WsF, bsF = fold(inputs['w_s'], inputs['b_s'], inputs['g_s'], inputs['be_s'])
    WyF, byF = fold(inputs['w_y'], inputs['b_y'], inputs['g_y'], inputs['be_y'])
    WoF, boF = fold(inputs['w_o'], inputs['b_o'], inputs['g_o'], inputs['be_o'])
    Wy2F, by2F = fold(inputs['w_y2'], inputs['b_y2'], inputs['g_y2'], inputs['be_y2'])
    by2FF = (Wy2F @ f(inputs['b3']) + by2F).astype(np.float32)

    # token-wise projections in fp32 on host (constant weights, ~1.5% of FLOPs)
    S1 = np.maximum(WsF @ Scm + bsF[None, :, None], 0.0)   # (B,48,4096)
    Y1 = np.maximum(WyF @ Ycm + byF[None, :, None], 0.0)   # (B,48,4096)
    scale = np.float32(Cs) ** np.float32(-0.5)
    Qt = np.einsum('de,bdn->ben', f(inputs['Wq']) * scale, Y1)   # (B,48,4096)
    Kt = np.einsum('de,bdn->ben', f(inputs['Wk']), Y1)           # (B,48,4096)
    Vt = np.einsum('de,bdn->ben', f(inputs['Wv']), S1)           # (B,48,4096)

    c = np.ascontiguousarray
    W3p = bf(f(inputs['w3']).reshape(Cy, Cy, 27).transpose(1, 2, 0))  # (i,t,o)
    B2 = c(np.stack([boF, by2FF], axis=1))                            # (48,2)

    Ypad = np.zeros((B, Cy, D + 2, H + 2, W + 2), np.float32)
    Ypad[:, :, 1:-1, 1:-1, 1:-1] = Ycm.reshape(B, Cy, D, H, W)

    # DW: w3 taps flat | Wy2F^T | WoF^T (padded to 96 rows)
    wo_pad = np.zeros((Cy, Cs), np.float32)
    wo_pad[:Cs] = WoF.T
    DW = bf(np.concatenate(
        [np.asarray(W3p, np.float32).reshape(Cy, 27 * Cy), Wy2F.T, wo_pad], axis=1))

    in_maps = []
    for core in range(NC):
        b, g = divmod(core, G)
        d0 = g * (D // G)
        sl = slice(g * KC, (g + 1) * KC)
        # D48: k | q | skb on 48 partitions, bf16
        D48 = bf(np.concatenate(
            [f(Kt[b, :, sl]), f(Qt[b]), f(Scm[b, :, sl])], axis=1))
        # DF: V flat [128, KT*Cs] | b2 [0:48, 384:386] f32
        Vk = c(Vt[b, :, sl].T.reshape(KT, 128, Cs).transpose(1, 0, 2))
        DF = np.zeros((128, KT * Cs + 2), np.float32)
        DF[:, :KT * Cs] = Vk.reshape(128, KT * Cs)
        DF[:Cs, KT * Cs:] = B2
        in_maps.append(dict(
            D48=D48,                                   # (48, 6144) bf16
            DF=c(DF),                                  # (128, 386) f32
            Yslab=bf(Ypad[b, :, d0:d0 + 6, :, :]),     # (96,6,18,18)
            DW=DW,                                     # (96, 2688) bf16
        ))
    return in_maps


# ---------------------------------------------------------------- bass build
def _build(repeat=1, ablate=()):
    """ablate: subset of {'rs','attn','conv'} — timing bisection only."""
    nc = bacc.Bacc("TRN2", target_bir_lowering=False, debug=False, num_devices=NC)

    D48 = nc.dram_tensor("D48", [Cs, KC + N + KC], BF16, kind="ExternalInput")
    DF = nc.dram_tensor("DF", [128, KT * Cs + 2], F32, kind="ExternalInput")
    Yslab = nc.dram_tensor("Yslab", [Cy, 6, 18, 18], BF16, kind="ExternalInput")
    DW = nc.dram_tensor("DW", [Cy, 27 * Cy + 2 * Cs], BF16, kind="ExternalInput")
    OUT = nc.dram_tensor("OUT", [2 * Cs, KC], F32, kind="ExternalOutput")

    with tile.TileContext(nc) as tc:
        with (
            tc.tile_pool(name="const", bufs=1) as cp,
            tc.tile_pool(name="data", bufs=1) as dp,
            tc.tile_pool(name="psum", bufs=2, space="PSUM") as pp,
            tc.tile_pool(name="dram", bufs=1, space="DRAM") as dram,
        ):
            def load(pool, t, shape, dt, tag=None, eng=None):
                s = pool.tile(shape, dt, tag=tag or t.name)
                (eng or nc.sync).dma_start(s[:], t.ap())
                return s

            # conv consts (on the ACT hwdge ring; sync ring carries D48)
            dw = load(cp, DW, [Cy, 27 * Cy + 2 * Cs], BF16, eng=nc.scalar)
            w3 = dw  # w3 tap t = dw[:, t*Cy:(t+1)*Cy]
            wy2 = dw[:, 27 * Cy:27 * Cy + Cs]           # (96,48)
            wo = dw[0:Cs, 27 * Cy + Cs:27 * Cy + 2 * Cs]  # (48,48)

            for rep in range(repeat):
                d48 = dp.tile([Cs, KC + N + KC], BF16, tag="d48")
                nc.sync.dma_start(d48[:, 0:KC + N // 2], D48.ap()[:, 0:KC + N // 2])
                nc.sync.dma_start(d48[:, KC + N // 2:], D48.ap()[:, KC + N // 2:])
                df = load(dp, DF, [128, KT * Cs + 2], F32, "df", eng=nc.scalar)
                # prime the ACT exp table while DMAs stream in
                # (scr produced on DVE at t~0: a DMA- or gpsimd-dependent
                # input would stall the strict-FIFO ACT queue and delay the
                # first real exp behind it)
                scr = dp.tile([Cs, 2], F32, tag="scr")
                nc.vector.memset(scr[:], 0.0)
                nc.scalar.activation(scr[:], scr[:], AF.Exp)
                yslab = load(dp, Yslab, [Cy, 6, 18, 18], BF16, "yslab",
                             eng=nc.scalar)

                e = dp.tile([128, KT, N], BF16, tag="e")
                z2 = dp.tile([128, KT, 4], F32, tag="z2")
                z = dp.tile([128, KT], F32, tag="z")
                zr = dp.tile([128, KT], F32, tag="zr")
                vp = dp.tile([128, KT, Cs], BF16, tag="vp")
                xstage = dp.tile([Cs, N], BF16, tag="xstage")

                if 'rs' not in ablate:
                    # tiny warmup RS: pre-heats the ncfw channel/credits under
                    # the exp phase so the real RS's control-plane is shorter
                    win = dram.tile([G * Cs, 4], FP8, tag="win")
                    wout = dram.tile([Cs, 4], FP8, tag="wout")
                    wsb = dp.tile([Cs, 4], FP8, tag="wsb")
                    nc.gpsimd.memset(wsb[:], 0.0)
                    for gg in range(G):
                        nc.sync.dma_start(win[gg * Cs:(gg + 1) * Cs, :], wsb[:])
                    nc.gpsimd.collective_compute(
                        "ReduceScatter", mybir.AluOpType.add,
                        replica_groups=[[0, 1, 2, 3], [4, 5, 6, 7]],
                        ins=[win[:]], outs=[wout[:]],
                    )

                attn_on = 'attn' not in ablate
                if not attn_on:
                    nc.gpsimd.memset(e[:], 0.25)
                    nc.gpsimd.memset(vp[:], 0.5)

                # ---- scores + exp + Z per k-tile
                for kt in range(KT if attn_on else 0):
                    lhs = d48[:, kt * 128:(kt + 1) * 128]
                    for h in range(N // SFD):
                        pss = pp.tile([128, SFD], F32, tag="ps")
                        for j in range(SFD // QC):
                            qs = slice(KC + h * SFD + j * QC,
                                       KC + h * SFD + (j + 1) * QC)
                            nc.tensor.matmul(pss[:, j * QC:(j + 1) * QC],
                                             lhs, d48[:, qs], start=True, stop=True)
                        nc.scalar.activation(
                            e[:, kt, h * SFD:(h + 1) * SFD], pss[:], AF.Exp,
                            accum_out=z2[:, kt, h:h + 1])
                    nc.vector.tensor_add(z[:, kt:kt + 1], z2[:, kt, 0:1],
                                         z2[:, kt, 1:2])
                    if kt == KT // 2 - 1 or kt == KT - 1:
                        kts = range(0, KT // 2) if kt < KT - 1 else range(KT // 2, KT)
                        nc.vector.reciprocal(zr[:, kts.start:kts.stop],
                                             z[:, kts.start:kts.stop])
                        for k2 in kts:
                            nc.vector.tensor_scalar_mul(
                                vp[:, k2, :], df[:, k2 * Cs:(k2 + 1) * Cs],
                                zr[:, k2:k2 + 1])

                # ---- x^T partials (kt-outer shares V' LDWEIGHTS; two
                # 2048-wide halves in one PSUM slot each)
                cin = dram.tile([G * Cs, KC], FP8, tag="cin")
                cout = dram.tile([Cs, KC], FP8, tag="cout")
                xs = dp.tile([Cs, N], FP8, tag="xs")
                for half in range(2):
                    psx = pp.tile([128, SFD], F32, tag="ps")
                    for kt in range(KT):
                        for j in range(SFD // QC):
                            es = slice(half * SFD + j * QC,
                                       half * SFD + (j + 1) * QC)
                            nc.tensor.matmul(psx[0:Cs, j * QC:(j + 1) * QC],
                                             vp[:, kt, :], e[:, kt, es],
                                             start=(kt == 0), stop=(kt == KT - 1))
                    if half == 0:
                        nc.scalar.copy(xs[:, half * SFD:(half + 1) * SFD],
                                       psx[0:Cs, :])
                    else:
                        nc.vector.tensor_copy(
                            xs[:, half * SFD:(half + 1) * SFD], psx[0:Cs, :])
                    for gg in range(2):
                        g4 = half * 2 + gg
                        eng = nc.sync if gg == 0 else nc.scalar
                        eng.dma_start(cin[g4 * Cs:(g4 + 1) * Cs, :],
                                      xs[:, g4 * KC:(g4 + 1) * KC])
                if 'rs' not in ablate:
                    nc.gpsimd.collective_compute(
                        "ReduceScatter", mybir.AluOpType.add,
                        replica_groups=[[0, 1, 2, 3], [4, 5, 6, 7]],
                        ins=[cin[:]], outs=[cout[:]],
                    )
                else:
                    nc.sync.dma_start(cout[:], cin[0:Cs, :])

                # ---- conv3x3x3 + Y2 (overlaps the RS waits; PE-only + DVE).
                # Both 512-token chunks share each tap's LDWEIGHTS: one
                # [128,2048] psum slot holds both accumulation regions.
                if 'conv' not in ablate:
                    psc = pp.tile([128, SFD], F32, tag="ps")
                    for t in range(27):
                        kd, r = divmod(t, 9)
                        kh, kw = divmod(r, 3)
                        for ci in range(2):
                            nc.tensor.matmul(
                                psc[0:Cy, ci * QC:(ci + 1) * QC],
                                w3[:, t * Cy:(t + 1) * Cy],
                                yslab[:, 2 * ci + kd:2 * ci + kd + 2,
                                      kh:kh + 16, kw:kw + 16],
                                start=(t == 0), stop=(t == 26))
                    c3 = dp.tile([Cy, KC], BF16, tag="c3")
                    nc.vector.tensor_copy(c3[:], psc[0:Cy, 0:KC])
                    psy = pp.tile([128, SFD], F32, tag="ps")
                    for ci in range(2):
                        nc.tensor.matmul(psy[0:Cs, ci * QC:(ci + 1) * QC],
                                         wy2,
                                         c3[:, ci * QC:(ci + 1) * QC],
                                         start=True, stop=True)
                    y2 = dp.tile([Cs, KC], F32, tag="y2")
                    nc.vector.tensor_scalar(y2[:], psy[0:Cs, 0:KC],
                                            df[0:Cs, KT * Cs + 1:KT * Cs + 2], 0.0,
                                            mybir.AluOpType.add,
                                            mybir.AluOpType.max)
                    nc.scalar.dma_start(OUT.ap()[Cs:2 * Cs, :], y2[:])

                # ---- post-RS: out-projection, mul by S+pe (xr halves on
                # both rings so proj[0] starts before half 1 lands)
                xr8 = dp.tile([Cs, KC], FP8, tag="xr8")
                nc.sync.dma_start(xr8[:, 0:QC], cout[0:Cs, 0:QC])
                nc.scalar.dma_start(xr8[:, QC:], cout[0:Cs, QC:])
                xr = dp.tile([Cs, KC], BF16, tag="xr")
                zo = dp.tile([Cs, KC], F32, tag="zo")
                for half in range(2):
                    sl = slice(half * QC, (half + 1) * QC)
                    if half == 0:
                        nc.scalar.copy(xr[:, sl], xr8[:, sl])
                    else:
                        nc.vector.tensor_copy(xr[:, sl], xr8[:, sl])
                    psz = pp.tile([128, SFD], F32, tag="ps")
                    nc.tensor.matmul(psz[0:Cs, 0:QC], wo, xr[:, sl],
                                     start=True, stop=True)
                    zc = dp.tile([Cs, QC], F32, tag=f"zc{half}")
                    nc.scalar.activation(zc[:], psz[0:Cs, 0:QC], AF.Relu,
                                         bias=df[0:Cs, KT * Cs:KT * Cs + 1])
                    nc.vector.tensor_mul(zo[:, sl], zc[:],
                                         d48[:, KC + N + half * QC:KC + N + (half + 1) * QC])
                    eng = nc.sync if half == 0 else nc.scalar
                    eng.dma_start(OUT.ap()[0:Cs, sl], zo[:, sl])

    nc.compile()
    return nc


class _Runner:
    """Builds the bass module once and a single reusable jitted callable
    (re-jitting per call would re-trace + re-hash the BIR module: ~600ms)."""

    def __init__(self, repeat=1, ablate=(), **kw):
        install_neuronx_cc_hook()
        nc = _build(repeat, ablate, **kw)
        self._setup_from_nc(nc)

    def _setup_from_nc(self, nc):
        install_neuronx_cc_hook()
        pid = nc.partition_id_tensor.name if nc.partition_id_tensor else None
        in_names, out_names, out_avals = [], [], []
        for alloc in nc.m.functions[0].allocations:
            if not isinstance(alloc, mybir.MemoryLocationSet):
                continue
            name = alloc.memorylocations[0].name
            if alloc.kind == "ExternalInput":
                if name != pid:
                    in_names.append(name)
            elif alloc.kind == "ExternalOutput":
                out_names.append(name)
                out_avals.append(jax.core.ShapedArray(
                    tuple(alloc.tensor_shape), mybir.dt.np(alloc.dtype)))
        self.in_names, self.out_names, self.out_avals = in_names, out_names, out_avals
        all_names = in_names + out_names + ([pid] if pid else [])

        def _body(*args):
            operands = list(args)
            if pid is not None:
                operands.append(bass2jax.partition_id_tensor())
            return tuple(_bass_exec_p.bind(
                *operands, out_avals=tuple(out_avals), in_names=tuple(all_names),
                out_names=tuple(out_names), lowering_input_output_aliases=(),
                sim_require_finite=True, sim_require_nnan=True, nc=nc))

        mesh = self.mesh = Mesh(np.asarray(jax.devices()[:NC]), ("core",))
        sp = (PartitionSpec("core"),)
        n_in = len(in_names) + len(out_names)
        self.fn = jax.jit(
            shard_map(_body, mesh=mesh, in_specs=sp * n_in,
                      out_specs=sp * len(out_names), check_rep=False),
            keep_unused=True)

    def device_args(self, in_maps):
        """Pre-stage all inputs on device (sharded) for low-overhead timed calls."""
        from jax.sharding import NamedSharding
        sh = NamedSharding(self.mesh, PartitionSpec("core"))
        cat = [np.concatenate([in_maps[c][n] for c in range(NC)], axis=0)
               for n in self.in_names]
        zz = [np.zeros((NC * a.shape[0], *a.shape[1:]), a.dtype)
              for a in self.out_avals]
        return [jax.device_put(a, sh) for a in cat + zz]

    def __call__(self, in_maps):
        outs = None
        for attempt in range(3):
            try:
                args = self.device_args(in_maps)
                jax.block_until_ready(args)  # staged before launch: less skew
                outs = self.fn(*args)
                jax.block_until_ready(outs)
                break
            except jax.errors.JaxRuntimeError:
                # transient tunnel hiccup (mesh desync / worker hangup):
                # restage and relaunch
                if attempt == 2:
                    raise
        assert outs is not None
        return [
            {n: np.asarray(outs[i]).reshape(NC, *self.out_avals[i].shape)[c]
             for i, n in enumerate(self.out_names)}
            for c in range(NC)
        ]


def _get(repeat=1, ablate=(), **kw):
    key = (repeat, tuple(sorted(ablate)), tuple(sorted(kw.items())))
    if key not in _cache:
        _cache[key] = _Runner(repeat, ablate, **kw)
    return _cache[key]


# ---------------------------------------------------------------- entry point
def kernel(**inputs):
    in_maps = _prepare(inputs)
    results = _get(1)(in_maps)
    out = np.zeros((B, 2 * Cs, D, H, W), np.float32)
    for core in range(NC):
        b, g = divmod(core, G)
        blk = results[core]["OUT"].reshape(2 * Cs, D // G, H, W)
        out[b, :, g * (D // G):(g + 1) * (D // G)] = blk
    return out


# revision 40
# speedup vs baseline: 1.1406x; 1.0903x over previous
"""Trainium2 Bass kernel for nn_MultiHeadCrossAttention_57638461112647.

Sharding: 8 cores = 2 batches x 4-way split over attention *keys* (and,
identically, over output tokens). The softmax in the reference is over the
*query* axis (axis=1), so with scores laid out (keys on partitions, queries on
free) the softmax denominator Z[k] is a free-axis row-sum, fully core-local.
The only cross-core exchange is one fp8 ReduceScatter of the attention
output partials x^T = V'^T E (48 x 4096) within each 4-core batch group
(fp8e4m3 partials measured 1.7e-3 output err vs the 2e-2 gate).

Host-side prep (cheap, O(N*C^2) ~ 1.5% of total FLOPs, all constant-weight
linear layers): fold BN-as-affine + biases into the 1x1-conv weights, add the
constant 3D positional encodings, compute the token-wise projections
S1/Y1/Q/K/V in fp32, fold the attention scale into Q, fold the 3x3x3-conv
bias through the following 1x1 conv, pad the conv input with its halo, and
slice per-core chunks. The device keeps the O(N^2) attention and the 3x3x3
conv (98.5% of FLOPs).

Device pipeline per core (channel-major layouts, channels on partitions):
  scores^T = K_own^T Q_all          (PE, 8 k-tiles of 128 x 4096 q, bf16)
  E = exp(scores) bf16              (ACT; no max-sub needed: scores O(0.2);
                                     Z[k] fused via accum_out per tile)
  V' = V/Z bf16 (DVE, per 4-tile group as Z completes)
  x^T partial = sum_kt V'[kt]^T E[kt]  (PE, bf16, kt-outer shares LDWEIGHTS)
  ReduceScatter(4-core group, fp8) -> x^T reduced for own 1024 tokens
  conv3x3x3 as 27 accumulating bf16 matmuls on a padded slab + 1x1 -> Y2
  (conv fills the RS wait; taps share LDWEIGHTS across both 512-chunks)
  OUT rows 0-47: relu(Wo x^T + b) * (S+pe)[own chunk]; rows 48-95: Y2
Inputs arrive as 4 packed DMAs split across the two HWDGE rings (sync +
act); exp table preloaded via a dummy activation during input DMA; a tiny
warmup ReduceScatter fires under the exp phase to pre-heat the ncfw channel
before the real collective; post-exp casts/relu run on the then-idle ACT
engine in parallel with DVE.
"""
import numpy as np
import ml_dtypes
import jax
from jax.sharding import Mesh, PartitionSpec
from jax.experimental.shard_map import shard_map

import concourse.bass as bass
import concourse.mybir as mybir
import concourse.tile as tile
from concourse import bacc
from concourse import bass2jax
from concourse.bass2jax import _bass_exec_p, install_neuronx_cc_hook

F32 = mybir.dt.float32
BF16 = mybir.dt.bfloat16
FP8 = mybir.dt.float8e4
AF = mybir.ActivationFunctionType
AX = mybir.AxisListType

B, Cy, Cs, D, H, W = 2, 96, 48, 16, 16, 16
N = D * H * W            # 4096 tokens
NC = 8                   # cores
G = 4                    # cores per batch
KC = N // G              # keys / output tokens per core = 1024
KT = KC // 128           # k-tiles per core = 8
QC = 512                 # free-dim chunk per matmul
SFD = 2048               # scores psum tile free dim (4 PSUM banks)
EPS = 1e-5

_cache = {}


# ---------------------------------------------------------------- host prep
def _pe3d(C, x, y, z):
    """Transcription of reference.pe3d (incl. its quirky torch broadcasting)."""
    c = int(np.ceil(C / 3))
    inv_freq = (1.0 / (10000.0 ** (np.arange(0, c, 2, dtype=np.float32) / c))
                ).astype(np.float32)

    def emb(n):
        s = np.arange(n, dtype=np.float32)[:, None] * inv_freq[None, :]
        return np.concatenate([np.sin(s), np.cos(s)], axis=-1).astype(np.float32)

    out = np.zeros((x, y, z, 3 * c), np.float32)
    out[..., :c] = emb(x)[:, None, :]        # broadcasts against (y, z, c)
    out[..., c:2 * c] = emb(y)[:, None, :]
    out[..., 2 * c:3 * c] = emb(z)
    return np.ascontiguousarray(out[..., :C].transpose(3, 0, 1, 2))  # (C,x,y,z)


def _prepare(inputs):
    f = lambda a: np.ascontiguousarray(np.asarray(a, np.float32))
    bf = lambda a: np.ascontiguousarray(np.asarray(a, ml_dtypes.bfloat16))
    Y, S = f(inputs['Y']), f(inputs['S'])

    pe_s = _pe3d(Cs, D, H, W).reshape(Cs, N)
    pe_y = _pe3d(Cy, D, H, W).reshape(Cy, N)
    Scm = S.reshape(B, Cs, N) + pe_s[None]          # (B,48,4096)
    Ycm = Y.reshape(B, Cy, N) + pe_y[None]          # (B,96,4096)

    sb = lambda g: f(g) / np.sqrt(np.float32(1.0) + np.float32(EPS))

    def fold(w, b, g, be):
        s = sb(g)
        return f(w) * s[:, None], (f(b) * s + f(be)).astype(np.float32)

    WsF, bsF = fold(inputs['w_s'], inputs['b_s'], inputs['g_s'], inputs['be_s'])
    WyF, byF = fold(inputs['w_y'], inputs['b_y'], inputs['g_y'], inputs['be_y'])
    WoF, boF = fold(inputs['w_o'], inputs['b_o'], inputs['g_o'], inputs['be_o'])
    Wy2F, by2F = fold(inputs['w_y2'], inputs['b_y2'], inputs['g_y2'], inputs['be_y2'])
    by2FF = (Wy2F @ f(inputs['b3']) + by2F).astype(np.float32)

    # token-wise projections in fp32 on host (constant weights, ~1.5% of FLOPs)
    S1 = np.maximum(WsF @ Scm + bsF[None, :, None], 0.0)   # (B,48,4096)
    Y1 = np.maximum(WyF @ Ycm + byF[None, :, None], 0.0)   # (B,48,4096)
    scale = np.float32(Cs) ** np.float32(-0.5)
    Qt = np.einsum('de,bdn->ben', f(inputs['Wq']) * scale, Y1)   # (B,48,4096)
    Kt = np.einsum('de,bdn->ben', f(inputs['Wk']), Y1)           # (B,48,4096)
    Vt = np.einsum('de,bdn->ben', f(inputs['Wv']), S1)           # (B,48,4096)

    c = np.ascontiguousarray
    W3p = bf(f(inputs['w3']).reshape(Cy, Cy, 27).transpose(1, 2, 0))  # (i,t,o)
    B2 = c(np.stack([boF, by2FF], axis=1))                            # (48,2)

    Ypad = np.zeros((B, Cy, D + 2, H + 2, W + 2), np.float32)
    Ypad[:, :, 1:-1, 1:-1, 1:-1] = Ycm.reshape(B, Cy, D, H, W)

    # DW: w3 taps flat | Wy2F^T | WoF^T (padded to 96 rows)
    wo_pad = np.zeros((Cy, Cs), np.float32)
    wo_pad[:Cs] = WoF.T
    DW = bf(np.concatenate(
        [np.asarray(W3p, np.float32).reshape(Cy, 27 * Cy), Wy2F.T, wo_pad], axis=1))

    in_maps = []
    for core in range(NC):
        b, g = divmod(core, G)
        d0 = g * (D // G)
        sl = slice(g * KC, (g + 1) * KC)
        # D48: k | q | skb on 48 partitions, bf16
        D48 = bf(np.concatenate(
            [f(Kt[b, :, sl]), f(Qt[b]), f(Scm[b, :, sl])], axis=1))
        # DF: V flat [128, KT*Cs] | b2 [0:48, 384:386] f32
        Vk = c(Vt[b, :, sl].T.reshape(KT, 128, Cs).transpose(1, 0, 2))
        DF = np.zeros((128, KT * Cs + 2), np.float32)
        DF[:, :KT * Cs] = Vk.reshape(128, KT * Cs)
        DF[:Cs, KT * Cs:] = B2
        in_maps.append(dict(
            D48=D48,                                   # (48, 6144) bf16
            DF=c(DF),                                  # (128, 386) f32
            Yslab=bf(Ypad[b, :, d0:d0 + 6, :, :]),     # (96,6,18,18)
            DW=DW,                                     # (96, 2688) bf16
        ))
    return in_maps


# ---------------------------------------------------------------- bass build
def _build(repeat=1, ablate=()):
    """ablate: subset of {'rs','attn','conv'} — timing bisection only."""
    nc = bacc.Bacc("TRN2", target_bir_lowering=False, debug=False, num_devices=NC)

    D48 = nc.dram_tensor("D48", [Cs, KC + N + KC], BF16, kind="ExternalInput")
    DF = nc.dram_tensor("DF", [128, KT * Cs + 2], F32, kind="ExternalInput")
    Yslab = nc.dram_tensor("Yslab", [Cy, 6, 18, 18], BF16, kind="ExternalInput")
    DW = nc.dram_tensor("DW", [Cy, 27 * Cy + 2 * Cs], BF16, kind="ExternalInput")
    OUT = nc.dram_tensor("OUT", [2 * Cs, KC], F32, kind="ExternalOutput")

    with tile.TileContext(nc) as tc:
        with (
            tc.tile_pool(name="const", bufs=1) as cp,
            tc.tile_pool(name="data", bufs=1) as dp,
            tc.tile_pool(name="psum", bufs=2, space="PSUM") as pp,
            tc.tile_pool(name="dram", bufs=1, space="DRAM") as dram,
        ):
            def load(pool, t, shape, dt, tag=None, eng=None):
                s = pool.tile(shape, dt, tag=tag or t.name)
                (eng or nc.sync).dma_start(s[:], t.ap())
                return s

            # conv consts (on the ACT hwdge ring; sync ring carries D48)
            dw = load(cp, DW, [Cy, 27 * Cy + 2 * Cs], BF16, eng=nc.scalar)
            w3 = dw  # w3 tap t = dw[:, t*Cy:(t+1)*Cy]
            wy2 = dw[:, 27 * Cy:27 * Cy + Cs]           # (96,48)
            wo = dw[0:Cs, 27 * Cy + Cs:27 * Cy + 2 * Cs]  # (48,48)

            for rep in range(repeat):
                d48 = dp.tile([Cs, KC + N + KC], BF16, tag="d48")
                nc.sync.dma_start(d48[:, 0:1536], D48.ap()[:, 0:1536])
                nc.sync.dma_start(d48[:, 1536:KC + N // 2],
                                  D48.ap()[:, 1536:KC + N // 2])
                nc.sync.dma_start(d48[:, KC + N // 2:], D48.ap()[:, KC + N // 2:])
                df = load(dp, DF, [128, KT * Cs + 2], F32, "df", eng=nc.scalar)
                # prime the ACT exp table while DMAs stream in
                # (scr produced on DVE at t~0: a DMA- or gpsimd-dependent
                # input would stall the strict-FIFO ACT queue and delay the
                # first real exp behind it)
                scr = dp.tile([Cs, 2], F32, tag="scr")
                nc.vector.memset(scr[:], 0.0)
                nc.scalar.activation(scr[:], scr[:], AF.Exp)
                yslab = load(dp, Yslab, [Cy, 6, 18, 18], BF16, "yslab",
                             eng=nc.scalar)

                e = dp.tile([128, KT, N], BF16, tag="e")
                z2 = dp.tile([128, KT, 4], F32, tag="z2")
                z = dp.tile([128, KT], F32, tag="z")
                zr = dp.tile([128, KT], F32, tag="zr")
                vp = dp.tile([128, KT, Cs], BF16, tag="vp")
                xstage = dp.tile([Cs, N], BF16, tag="xstage")

                if 'rs' not in ablate:
                    # tiny warmup RS: pre-heats the ncfw channel/credits under
                    # the exp phase so the real RS's control-plane is shorter
                    win = dram.tile([G * Cs, 4], FP8, tag="win")
                    wout = dram.tile([Cs, 4], FP8, tag="wout")
                    wsb = dp.tile([Cs, 4], FP8, tag="wsb")
                    nc.gpsimd.memset(wsb[:], 0.0)
                    for gg in range(G):
                        nc.sync.dma_start(win[gg * Cs:(gg + 1) * Cs, :], wsb[:])
                    nc.gpsimd.collective_compute(
                        "ReduceScatter", mybir.AluOpType.add,
                        replica_groups=[[0, 1, 2, 3], [4, 5, 6, 7]],
                        ins=[win[:]], outs=[wout[:]],
                    )

                attn_on = 'attn' not in ablate
                if not attn_on:
                    nc.gpsimd.memset(e[:], 0.25)
                    nc.gpsimd.memset(vp[:], 0.5)

                # ---- scores + exp + Z per k-tile
                for kt in range(KT if attn_on else 0):
                    lhs = d48[:, kt * 128:(kt + 1) * 128]
                    for h in range(N // SFD):
                        pss = pp.tile([128, SFD], F32, tag="ps")
                        for j in range(SFD // QC):
                            qs = slice(KC + h * SFD + j * QC,
                                       KC + h * SFD + (j + 1) * QC)
                            nc.tensor.matmul(pss[:, j * QC:(j + 1) * QC],
                                             lhs, d48[:, qs], start=True, stop=True)
                        nc.scalar.activation(
                            e[:, kt, h * SFD:(h + 1) * SFD], pss[:], AF.Exp,
                            accum_out=z2[:, kt, h:h + 1])
                    nc.vector.tensor_add(z[:, kt:kt + 1], z2[:, kt, 0:1],
                                         z2[:, kt, 1:2])
                    if kt == KT // 2 - 1 or kt == KT - 1:
                        kts = range(0, KT // 2) if kt < KT - 1 else range(KT // 2, KT)
                        nc.vector.reciprocal(zr[:, kts.start:kts.stop],
                                             z[:, kts.start:kts.stop])
                        for k2 in kts:
                            nc.vector.tensor_scalar_mul(
                                vp[:, k2, :], df[:, k2 * Cs:(k2 + 1) * Cs],
                                zr[:, k2:k2 + 1])

                # ---- x^T partials (kt-outer shares V' LDWEIGHTS; two
                # 2048-wide halves in one PSUM slot each)
                cin = dram.tile([G * Cs, KC], FP8, tag="cin")
                cout = dram.tile([Cs, KC], FP8, tag="cout")
                xs = dp.tile([Cs, N], FP8, tag="xs")
                for half in range(2):
                    psx = pp.tile([128, SFD], F32, tag="ps")
                    for kt in range(KT):
                        for j in range(SFD // QC):
                            es = slice(half * SFD + j * QC,
                                       half * SFD + (j + 1) * QC)
                            nc.tensor.matmul(psx[0:Cs, j * QC:(j + 1) * QC],
                                             vp[:, kt, :], e[:, kt, es],
                                             start=(kt == 0), stop=(kt == KT - 1))
                    if half == 0:
                        nc.scalar.copy(xs[:, half * SFD:(half + 1) * SFD],
                                       psx[0:Cs, :])
                    else:
                        nc.vector.tensor_copy(
                            xs[:, half * SFD:(half + 1) * SFD], psx[0:Cs, :])
                    for gg in range(2):
                        g4 = half * 2 + gg
                        eng = nc.sync if gg == 0 else nc.scalar
                        eng.dma_start(cin[g4 * Cs:(g4 + 1) * Cs, :],
                                      xs[:, g4 * KC:(g4 + 1) * KC])
                if 'rs' not in ablate:
                    nc.gpsimd.collective_compute(
                        "ReduceScatter", mybir.AluOpType.add,
                        replica_groups=[[0, 1, 2, 3], [4, 5, 6, 7]],
                        ins=[cin[:]], outs=[cout[:]],
                    )
                else:
                    nc.sync.dma_start(cout[:], cin[0:Cs, :])

                # ---- conv3x3x3 + Y2 (overlaps the RS waits; PE-only + DVE).
                # Both 512-token chunks share each tap's LDWEIGHTS: one
                # [128,2048] psum slot holds both accumulation regions.
                if 'conv' not in ablate:
                    psc = pp.tile([128, SFD], F32, tag="ps")
                    for t in range(27):
                        kd, r = divmod(t, 9)
                        kh, kw = divmod(r, 3)
                        for ci in range(2):
                            nc.tensor.matmul(
                                psc[0:Cy, ci * QC:(ci + 1) * QC],
                                w3[:, t * Cy:(t + 1) * Cy],
                                yslab[:, 2 * ci + kd:2 * ci + kd + 2,
                                      kh:kh + 16, kw:kw + 16],
                                start=(t == 0), stop=(t == 26))
                    c3 = dp.tile([Cy, KC], BF16, tag="c3")
                    nc.vector.tensor_copy(c3[:], psc[0:Cy, 0:KC])
                    psy = pp.tile([128, SFD], F32, tag="ps")
                    for ci in range(2):
                        nc.tensor.matmul(psy[0:Cs, ci * QC:(ci + 1) * QC],
                                         wy2,
                                         c3[:, ci * QC:(ci + 1) * QC],
                                         start=True, stop=True)
                    y2 = dp.tile([Cs, KC], F32, tag="y2")
                    nc.vector.tensor_scalar(y2[:], psy[0:Cs, 0:KC],
                                            df[0:Cs, KT * Cs + 1:KT * Cs + 2], 0.0,
                                            mybir.AluOpType.add,
                                            mybir.AluOpType.max)
                    nc.scalar.dma_start(OUT.ap()[Cs:2 * Cs, :], y2[:])

                # ---- post-RS: out-projection, mul by S+pe (xr halves on
                # both rings so proj[0] starts before half 1 lands)
                xr8 = dp.tile([Cs, KC], FP8, tag="xr8")
                nc.sync.dma_start(xr8[:, 0:QC], cout[0:Cs, 0:QC])
                nc.scalar.dma_start(xr8[:, QC:], cout[0:Cs, QC:])
                xr = dp.tile([Cs, KC], BF16, tag="xr")
                zo = dp.tile([Cs, KC], F32, tag="zo")
                for half in range(2):
                    sl = slice(half * QC, (half + 1) * QC)
                    if half == 0:
                        nc.scalar.copy(xr[:, sl], xr8[:, sl])
                    else:
                        nc.vector.tensor_copy(xr[:, sl], xr8[:, sl])
                    psz = pp.tile([128, SFD], F32, tag="ps")
                    nc.tensor.matmul(psz[0:Cs, 0:QC], wo, xr[:, sl],
                                     start=True, stop=True)
                    zc = dp.tile([Cs, QC], F32, tag=f"zc{half}")
                    nc.scalar.activation(zc[:], psz[0:Cs, 0:QC], AF.Relu,
                                         bias=df[0:Cs, KT * Cs:KT * Cs + 1])
                    nc.vector.tensor_mul(zo[:, sl], zc[:],
                                         d48[:, KC + N + half * QC:KC + N + (half + 1) * QC])
                    eng = nc.sync if half == 0 else nc.scalar
                    eng.dma_start(OUT.ap()[0:Cs, sl], zo[:, sl])

    nc.compile()
    return nc


class _Runner:
    """Builds the bass module once and a single reusable jitted callable
    (re-jitting per call would re-trace + re-hash the BIR module: ~600ms)."""

    def __init__(self, repeat=1, ablate=(), **kw):
        install_neuronx_cc_hook()
        nc = _build(repeat, ablate, **kw)
        self._setup_from_nc(nc)

    def _setup_from_nc(self, nc):
        install_neuronx_cc_hook()
        pid = nc.partition_id_tensor.name if nc.partition_id_tensor else None
        in_names, out_names, out_avals = [], [], []
        for alloc in nc.m.functions[0].allocations:
            if not isinstance(alloc, mybir.MemoryLocationSet):
                continue
            name = alloc.memorylocations[0].name
            if alloc.kind == "ExternalInput":
                if name != pid:
                    in_names.append(name)
            elif alloc.kind == "ExternalOutput":
                out_names.append(name)
                out_avals.append(jax.core.ShapedArray(
                    tuple(alloc.tensor_shape), mybir.dt.np(alloc.dtype)))
        self.in_names, self.out_names, self.out_avals = in_names, out_names, out_avals
        all_names = in_names + out_names + ([pid] if pid else [])

        def _body(*args):
            operands = list(args)
            if pid is not None:
                operands.append(bass2jax.partition_id_tensor())
            return tuple(_bass_exec_p.bind(
                *operands, out_avals=tuple(out_avals), in_names=tuple(all_names),
                out_names=tuple(out_names), lowering_input_output_aliases=(),
                sim_require_finite=True, sim_require_nnan=True, nc=nc))

        mesh = self.mesh = Mesh(np.asarray(jax.devices()[:NC]), ("core",))
        sp = (PartitionSpec("core"),)
        n_in = len(in_names) + len(out_names)
        self.fn = jax.jit(
            shard_map(_body, mesh=mesh, in_specs=sp * n_in,
                      out_specs=sp * len(out_names), check_rep=False),
            keep_unused=True)

    def device_args(self, in_maps):
        """Pre-stage all inputs on device (sharded) for low-overhead timed calls."""
        from jax.sharding import NamedSharding
        sh = NamedSharding(self.mesh, PartitionSpec("core"))
        cat = [np.concatenate([in_maps[c][n] for c in range(NC)], axis=0)
               for n in self.in_names]
        zz = [np.zeros((NC * a.shape[0], *a.shape[1:]), a.dtype)
              for a in self.out_avals]
        return [jax.device_put(a, sh) for a in cat + zz]

    def __call__(self, in_maps):
        outs = None
        for attempt in range(3):
            try:
                args = self.device_args(in_maps)
                jax.block_until_ready(args)  # staged before launch: less skew
                outs = self.fn(*args)
                jax.block_until_ready(outs)
                break
            except jax.errors.JaxRuntimeError:
                # transient tunnel hiccup (mesh desync / worker hangup):
                # restage and relaunch
                if attempt == 2:
                    raise
        assert outs is not None
        return [
            {n: np.asarray(outs[i]).reshape(NC, *self.out_avals[i].shape)[c]
             for i, n in enumerate(self.out_names)}
            for c in range(NC)
        ]


def _get(repeat=1, ablate=(), **kw):
    key = (repeat, tuple(sorted(ablate)), tuple(sorted(kw.items())))
    if key not in _cache:
        _cache[key] = _Runner(repeat, ablate, **kw)
    return _cache[key]


# ---------------------------------------------------------------- entry point
def kernel(**inputs):
    in_maps = _prepare(inputs)
    results = _get(1)(in_maps)
    out = np.zeros((B, 2 * Cs, D, H, W), np.float32)
    for core in range(NC):
        b, g = divmod(core, G)
        blk = results[core]["OUT"].reshape(2 * Cs, D // G, H, W)
        out[b, :, g * (D // G):(g + 1) * (D // G)] = blk
    return out
